# revision 1
# baseline (speedup 1.0000x reference)
"""Nearest-E8-lattice quantizer (CachedE8Quantizer) as a Bass/Tile kernel on 8 trn2 cores.

Input x: [8388608, 8] fp32. Output: nearest point of E8 = D8 u (D8 + 1/2).

Sharding: data-parallel over the points dim, 1/8 per core (no comms).

Per-core pipeline, layout [128 partitions, TF points, 8 coords] per tile:
  r0  = round(x)            via (x + 1.5*2^23) - 1.5*2^23   (exact RNE)   [GP]
  r1h = round(x - 0.5)+0.5  via ((x-0.5)+C) - C + 0.5                     [GP]
  d_b = x - r_b   (exact, Sterbenz)                                       [DVE]
  q_b = d_b^2                                                             [ACT]
  segmented (per 8) reduces: s_b = sum r_b, S2_b = sum q_b, mq_b = max q_b [DVE]
  parity p_b of s_b; u_b = 1 - 2*sqrt(mq_b); D_b = S2_b + p_b*u_b;
  c = D0 <= D1; w_b = p_b * (c match); mgq_b = mq_b*w + (w-1)   (q or -1) [smalls]
  onehot = (q_b == mgq_b)  fp equality (argmax coord; rare sq-ties double-flip) [GP]
  flip_b = signbit(d_b) | onehot-bits   (+-1.0f at argmax, +-0.0 elsewhere) [DVE]
  y = (x - (c ? d0 : d1)) + flip0 + flip1                                  [DVE+GP]
"""

import numpy as np

from concourse import bacc
import concourse.mybir as mybir
from concourse.alu_op_type import AluOpType as op
from concourse.bass_utils import run_bass_kernel_spmd
from concourse.tile import TileContext

N_POINTS = 8388608
N_CORES = 8
SHARD = N_POINTS // N_CORES  # 1048576 points per core

MAGIC = 12582912.0  # 1.5 * 2**23: (x + MAGIC) - MAGIC == round-half-even(x)
F32 = mybir.dt.float32
U32 = mybir.dt.uint32
X = mybir.AxisListType.X


def _stt_u32(eng, out, in0, scalar_int, in1, op0, op1):
    """scalar_tensor_tensor with a uint32 immediate (bass default lowers ints
    as f32 immediates, which walrus rejects for bitvec ops)."""
    return eng.add_instruction(
        mybir.InstTensorScalarPtr(
            name=eng.bass.get_next_instruction_name(),
            is_scalar_tensor_tensor=True,
            op0=op0,
            op1=op1,
            ins=[
                eng.lower_ap(in0),
                mybir.ImmediateValue(dtype=U32, value=scalar_int),
                eng.lower_ap(in1),
            ],
            outs=[eng.lower_ap(out)],
        )
    )


ENGINES = {"round": "vector", "flsum": "vector", "delta": "vector", "round_act": False, "pe_y": True, "pe_d": False, "pe_qd": False}


def _emit_tile(nc, pools, xd, yd, t, tf):
    E = lambda k: getattr(nc, ENGINES[k])
    P = 128
    pts = P * tf
    FE = tf * 8
    stream, work, small = pools[:3]

    s = t * pts
    x_rows = xd[s : s + pts, :].rearrange("(p f) c -> p (f c)", p=P)
    y_rows = yd[s : s + pts, :].rearrange("(p f) c -> p (f c)", p=P)

    xt = stream.tile([P, FE], F32, tag="xt")
    nc.sync.dma_start(out=xt[:], in_=x_rows)

    # roundings on GPSIMD; t1 scratch shares the ohh slot
    rr = work.tile([P, 2 * FE], F32, tag="rr")
    t1 = work.tile([P, 2 * FE], F32, tag="ohh")
    r0, r1h = rr[:, :FE], rr[:, FE:]
    CP = mybir.ActivationFunctionType.Copy
    if ENGINES["round_act"]:
        nc.scalar.activation(r0, xt[:], CP, bias=MAGIC)
        nc.scalar.activation(r0, r0, CP, bias=-MAGIC)
        nc.scalar.activation(t1[:, :FE], xt[:], CP, bias=-0.5)
        nc.scalar.activation(t1[:, :FE], t1[:, :FE], CP, bias=MAGIC)
        nc.scalar.activation(r1h, t1[:, :FE], CP, bias=-MAGIC)
        nc.scalar.activation(r1h, r1h, CP, bias=0.5)
    else:
        E("round").tensor_scalar(r0, xt[:], MAGIC, MAGIC, op0=op.add, op1=op.subtract)
        E("round").tensor_scalar(t1[:, :FE], xt[:], 0.5, MAGIC, op0=op.subtract, op1=op.add)
        E("round").tensor_scalar(r1h, t1[:, :FE], MAGIC, 0.5, op0=op.subtract, op1=op.add)

    # deltas
    dd = work.tile([P, 2 * FE], F32, tag="dd")
    d0, d1 = dd[:, :FE], dd[:, FE:]
    if ENGINES["pe_d"]:
        psum_pool, ident, nident = pools[3]
        NCH = 512
        dp = psum_pool.tile([P, 2 * FE], F32, tag="dp")
        for c0 in range(0, FE, NCH):
            nc.tensor.matmul(dp[:, c0:c0+NCH], ident[:], xt[:, c0:c0+NCH], start=True, stop=False)
            nc.tensor.matmul(dp[:, c0:c0+NCH], nident[:], rr[:, c0:c0+NCH], start=False, stop=True)
            nc.tensor.matmul(dp[:, FE+c0:FE+c0+NCH], ident[:], xt[:, c0:c0+NCH], start=True, stop=False)
            nc.tensor.matmul(dp[:, FE+c0:FE+c0+NCH], nident[:], rr[:, FE+c0:FE+c0+NCH], start=False, stop=True)
        nc.scalar.copy(dd[:, :FE], dp[:, :FE])
        nc.scalar.copy(dd[:, FE:], dp[:, FE:])
    else:
        E("delta").tensor_tensor(d0, xt[:], r0, op.subtract)
        E("delta").tensor_tensor(d1, xt[:], r1h, op.subtract)
    dd_u = dd[:].bitcast(U32)

    # squares (ACT)
    qq = work.tile([P, 2 * FE], F32, tag="qq")
    nc.scalar.square(qq[:, :FE], d0)
    nc.scalar.square(qq[:, FE:], d1)
    qq3 = qq[:].rearrange("p (t c) -> p t c", c=8)

    # segmented reduces (DVE)
    rr3 = rr[:].rearrange("p (t c) -> p t c", c=8)
    TW = 2 * tf
    arena = small.tile([P, 8 * TW + tf], F32, tag="arena")
    savg = arena[:, 0 * TW : 1 * TW]
    qavg = arena[:, 1 * TW : 2 * TW]
    mq2 = arena[:, 2 * TW : 3 * TW]
    ps2 = arena[:, 3 * TW : 4 * TW]
    p2f = arena[:, 4 * TW : 5 * TW]
    u2 = arena[:, 5 * TW : 6 * TW]
    Dv2 = arena[:, 6 * TW : 7 * TW]  # also reused as vg scratch
    wf2 = arena[:, 7 * TW : 8 * TW]
    cf = arena[:, 8 * TW : 8 * TW + tf]
    nc.vector.tensor_reduce(savg, rr3, axis=X, op=op.add)
    if ENGINES["pe_qd"]:
        psum_pool2, ident2, nident2 = pools[3]
        NCH = 512
        qdp = psum_pool2.tile([P, FE], F32, tag="qdp")
        for c0 in range(0, FE, NCH):
            nc.tensor.matmul(qdp[:, c0:c0+NCH], ident2[:], qq[:, c0:c0+NCH], start=True, stop=False)
            nc.tensor.matmul(qdp[:, c0:c0+NCH], nident2[:], qq[:, FE+c0:FE+c0+NCH], start=False, stop=True)
        qdp3 = qdp[:].rearrange("p (t c) -> p t c", c=8)
        nc.vector.tensor_reduce(qavg[:, :tf], qdp3, axis=X, op=op.add)  # dS = S2_0 - S2_1
    else:
        nc.vector.tensor_reduce(qavg, qq3, axis=X, op=op.add)
    nc.vector.tensor_reduce(mq2, qq3, axis=X, op=op.max)

    # parity: ps = 2*round(s/2) - s in {-1,0,1}; p2f = ps^2 in {0,1}
    nc.vector.tensor_scalar(ps2, savg, 0.5, MAGIC, op0=op.mult, op1=op.add)
    nc.vector.tensor_scalar(ps2, ps2, MAGIC, None, op0=op.subtract)
    nc.vector.scalar_tensor_tensor(ps2, ps2, 2.0, savg, op0=op.mult, op1=op.subtract)
    nc.scalar.square(p2f, ps2)
    # u = 1 - 2*sqrt(mq)  (~1ulp sqrt; only perturbs borderline D compares)
    nc.scalar.sqrt(u2, mq2)
    nc.scalar.activation(
        u2, u2, mybir.ActivationFunctionType.Copy, bias=1.0, scale=-2.0
    )
    # D = S2 + p*u ; c = (D0 <= D1)
    nc.vector.tensor_tensor(Dv2, p2f, u2, op.mult)
    if ENGINES["pe_qd"]:
        nc.vector.tensor_tensor(Dv2[:, :tf], Dv2[:, :tf], qavg[:, :tf], op.add)
        nc.vector.tensor_tensor(cf, Dv2[:, :tf], Dv2[:, tf:], op.is_le)
    else:
        nc.vector.tensor_tensor(Dv2, Dv2, qavg, op.add)
        nc.vector.tensor_tensor(cf, Dv2[:, :tf], Dv2[:, tf:], op.is_le)
    # w0 = p0*c ; w1 = p1*(1-c); gated max-sq: mgq = mq*w + (w-1)
    nc.vector.tensor_tensor(wf2[:, :tf], p2f[:, :tf], cf, op.mult)
    nc.vector.scalar_tensor_tensor(
        wf2[:, tf:], cf, 1.0, p2f[:, tf:], op0=op.subtract, op1=op.mult
    )
    nc.vector.tensor_scalar(wf2[:, tf:], wf2[:, tf:], -1.0, None, op0=op.mult)
    nc.vector.tensor_tensor(Dv2, mq2, wf2, op.mult)
    nc.vector.tensor_scalar(wf2, wf2, 1.0, None, op0=op.subtract)
    nc.vector.tensor_tensor(mq2, Dv2, wf2, op.add)

    # onehot (GP, fp equality on squares); flip = sign(d) | onehot-bits (DVE)
    mgq_b = mq2.unsqueeze(2).broadcast_to([P, 2 * tf, 8])
    ohf = work.tile([P, 2 * FE], F32, tag="ohh")
    ohf3 = ohf[:].rearrange("p (t c) -> p t c", c=8)
    nc.vector.tensor_tensor(ohf3, qq3, mgq_b, op.is_equal)
    ohf_u = ohf[:].bitcast(U32)
    _stt_u32(nc.vector, ohf_u, dd_u[:], 0x80000000, ohf_u, op.bitwise_and, op.bitwise_or)
    fl = ohf[:]

    # d_sel = c ? d0 : d1 (ACT copy + DVE predicated); flsum on GP
    dsel = work.tile([P, 2 * FE], F32, tag="rr")
    nc.scalar.copy(dsel[:, :FE], d1)
    cI_b = cf.bitcast(U32).unsqueeze(2).broadcast_to([P, tf, 8])
    nc.vector.copy_predicated(
        dsel[:, :FE].rearrange("p (t c) -> p t c", c=8),
        cI_b,
        dd[:, :FE].rearrange("p (t c) -> p t c", c=8),
    )
    if ENGINES["pe_y"]:
        psum_pool, ident, nident = pools[3]
        yp = psum_pool.tile([P, FE], F32, tag="yp")
        NCH = 512
        for c0 in range(0, FE, NCH):
            sl = slice(c0, c0 + NCH)
            nc.tensor.matmul(yp[:, sl], ident[:], xt[:, sl], start=True, stop=False)
            nc.tensor.matmul(yp[:, sl], nident[:], dsel[:, c0:c0+NCH], start=False, stop=False)
            nc.tensor.matmul(yp[:, sl], ident[:], fl[:, c0:c0+NCH], start=False, stop=False)
            nc.tensor.matmul(yp[:, sl], ident[:], fl[:, FE+c0:FE+c0+NCH], start=False, stop=True)
        yt = stream.tile([P, FE], F32, tag="yt")
        nc.scalar.copy(yt[:], yp[:])
        nc.sync.dma_start(out=y_rows, in_=yt[:])
    else:
        E("flsum").tensor_tensor(dsel[:, FE:], fl[:, :FE], fl[:, FE:], op.add)
        yt = stream.tile([P, FE], F32, tag="yt")
        nc.vector.tensor_tensor(yt[:], xt[:], dsel[:, :FE], op.subtract)
        nc.vector.tensor_tensor(yt[:], yt[:], dsel[:, FE:], op.add)
        nc.sync.dma_start(out=y_rows, in_=yt[:])


def build_nc(shard=SHARD, tf=256):
    P = 128
    pts = P * tf
    assert shard % pts == 0
    ntiles = shard // pts

    nc = bacc.Bacc("TRN2", target_bir_lowering=False, debug=False, num_devices=N_CORES)
    xd = nc.declare_dram_parameter("x", [shard, 8], F32, isOutput=False)
    yd = nc.declare_dram_parameter("y", [shard, 8], F32, isOutput=True)

    from concourse.masks import make_identity
    with TileContext(nc) as tc:
        with (
            tc.tile_pool(name="stream", bufs=2) as stream,
            tc.tile_pool(name="work", bufs=2) as work,
            tc.tile_pool(name="small", bufs=2) as small,
            tc.tile_pool(name="const", bufs=1) as cpool,
            tc.tile_pool(name="psum", bufs=2, space="PSUM") as psum_pool,
        ):
            pe = None
            if ENGINES["pe_y"]:
                ident = cpool.tile([P, P], F32, tag="ident")
                nident = cpool.tile([P, P], F32, tag="nident")
                make_identity(nc, ident[:])
                nc.scalar.activation(
                    nident[:], ident[:], mybir.ActivationFunctionType.Copy, scale=-1.0
                )
                pe = (psum_pool, ident, nident)
            for t in range(ntiles):
                _emit_tile(nc, (stream, work, small, pe), xd, yd, t, tf)
    nc.finalize()
    return nc


_BUILD_CACHE = {}


def _get_nc(shard, tf):
    key = (shard, tf)
    if key not in _BUILD_CACHE:
        _BUILD_CACHE[key] = build_nc(shard, tf)
    return _BUILD_CACHE[key]


def kernel(x: np.ndarray) -> np.ndarray:
    x = np.ascontiguousarray(x, dtype=np.float32)
    n = x.shape[0]
    shard = n // N_CORES
    tf = 256
    while shard % (128 * tf) != 0:
        tf //= 2
    nc = _get_nc(shard, tf)
    in_maps = [{"x": x[i * shard : (i + 1) * shard]} for i in range(N_CORES)]
    res = run_bass_kernel_spmd(nc, in_maps, list(range(N_CORES))).results
    return np.concatenate([res[i]["y"] for i in range(N_CORES)], axis=0)



# revision 3
# speedup vs baseline: 905.2275x; 905.2275x over previous
"""Nearest-E8-lattice quantizer (CachedE8Quantizer) as a Bass/Tile kernel on 8 trn2 cores.

Input x: [8388608, 8] fp32. Output: nearest point of E8 = D8 u (D8 + 1/2).

Sharding: data-parallel over the points dim, 1/8 per core (no comms).

Per-core pipeline, layout [128 partitions, TF points, 8 coords] per tile:
  r0  = round(x)            via (x + 1.5*2^23) - 1.5*2^23   (exact RNE)   [GP]
  r1h = round(x - 0.5)+0.5  via ((x-0.5)+C) - C + 0.5                     [GP]
  d_b = x - r_b   (exact, Sterbenz)                                       [DVE]
  q_b = d_b^2                                                             [ACT]
  segmented (per 8) reduces: s_b = sum r_b, S2_b = sum q_b, mq_b = max q_b [DVE]
  parity p_b of s_b; u_b = 1 - 2*sqrt(mq_b); D_b = S2_b + p_b*u_b;
  c = D0 <= D1; w_b = p_b * (c match); mgq_b = mq_b*w + (w-1)   (q or -1) [smalls]
  onehot = (q_b == mgq_b)  fp equality (argmax coord; rare sq-ties double-flip) [GP]
  flip_b = signbit(d_b) | onehot-bits   (+-1.0f at argmax, +-0.0 elsewhere) [DVE]
  y = (x - (c ? d0 : d1)) + flip0 + flip1                                  [DVE+GP]
"""

import numpy as np

from concourse import bacc
import concourse.mybir as mybir
from concourse.alu_op_type import AluOpType as op
from concourse.bass_utils import run_bass_kernel_spmd
from concourse.tile import TileContext

N_POINTS = 8388608
N_CORES = 8
SHARD = N_POINTS // N_CORES  # 1048576 points per core

MAGIC = 12582912.0  # 1.5 * 2**23: (x + MAGIC) - MAGIC == round-half-even(x)
F32 = mybir.dt.float32
U32 = mybir.dt.uint32
X = mybir.AxisListType.X


def _stt_u32(eng, out, in0, scalar_int, in1, op0, op1):
    """scalar_tensor_tensor with a uint32 immediate (bass default lowers ints
    as f32 immediates, which walrus rejects for bitvec ops)."""
    return eng.add_instruction(
        mybir.InstTensorScalarPtr(
            name=eng.bass.get_next_instruction_name(),
            is_scalar_tensor_tensor=True,
            op0=op0,
            op1=op1,
            ins=[
                eng.lower_ap(in0),
                mybir.ImmediateValue(dtype=U32, value=scalar_int),
                eng.lower_ap(in1),
            ],
            outs=[eng.lower_ap(out)],
        )
    )


ENGINES = {"round": "vector", "flsum": "vector", "delta": "vector", "round_act": False, "pe_y": True, "pe_d": False, "pe_qd": False}


def _emit_tile(nc, pools, xd, yd, t, tf):
    E = lambda k: getattr(nc, ENGINES[k])
    P = 128
    pts = P * tf
    FE = tf * 8
    stream, work, small = pools[:3]

    s = t * pts
    x_rows = xd[s : s + pts, :].rearrange("(p f) c -> p (f c)", p=P)
    y_rows = yd[s : s + pts, :].rearrange("(p f) c -> p (f c)", p=P)

    xt = stream.tile([P, FE], F32, tag="xt")
    nc.sync.dma_start(out=xt[:], in_=x_rows)

    # roundings on GPSIMD; t1 scratch shares the ohh slot
    rr = work.tile([P, 2 * FE], F32, tag="rr")
    t1 = work.tile([P, 2 * FE], F32, tag="ohh")
    r0, r1h = rr[:, :FE], rr[:, FE:]
    CP = mybir.ActivationFunctionType.Copy
    if ENGINES["round_act"]:
        nc.scalar.activation(r0, xt[:], CP, bias=MAGIC)
        nc.scalar.activation(r0, r0, CP, bias=-MAGIC)
        nc.scalar.activation(t1[:, :FE], xt[:], CP, bias=-0.5)
        nc.scalar.activation(t1[:, :FE], t1[:, :FE], CP, bias=MAGIC)
        nc.scalar.activation(r1h, t1[:, :FE], CP, bias=-MAGIC)
        nc.scalar.activation(r1h, r1h, CP, bias=0.5)
    else:
        E("round").tensor_scalar(r0, xt[:], MAGIC, MAGIC, op0=op.add, op1=op.subtract)
        E("round").tensor_scalar(t1[:, :FE], xt[:], 0.5, MAGIC, op0=op.subtract, op1=op.add)
        E("round").tensor_scalar(r1h, t1[:, :FE], MAGIC, 0.5, op0=op.subtract, op1=op.add)

    # deltas
    dd = work.tile([P, 2 * FE], F32, tag="dd")
    d0, d1 = dd[:, :FE], dd[:, FE:]
    if ENGINES["pe_d"]:
        psum_pool, ident, nident = pools[3]
        NCH = 512
        dp = psum_pool.tile([P, 2 * FE], F32, tag="dp")
        for c0 in range(0, FE, NCH):
            nc.tensor.matmul(dp[:, c0:c0+NCH], ident[:], xt[:, c0:c0+NCH], start=True, stop=False)
            nc.tensor.matmul(dp[:, c0:c0+NCH], nident[:], rr[:, c0:c0+NCH], start=False, stop=True)
            nc.tensor.matmul(dp[:, FE+c0:FE+c0+NCH], ident[:], xt[:, c0:c0+NCH], start=True, stop=False)
            nc.tensor.matmul(dp[:, FE+c0:FE+c0+NCH], nident[:], rr[:, FE+c0:FE+c0+NCH], start=False, stop=True)
        nc.scalar.copy(dd[:, :FE], dp[:, :FE])
        nc.scalar.copy(dd[:, FE:], dp[:, FE:])
    else:
        E("delta").tensor_tensor(d0, xt[:], r0, op.subtract)
        E("delta").tensor_tensor(d1, xt[:], r1h, op.subtract)
    dd_u = dd[:].bitcast(U32)

    # squares (ACT)
    qq = work.tile([P, 2 * FE], F32, tag="qq")
    nc.scalar.square(qq[:, :FE], d0)
    nc.scalar.square(qq[:, FE:], d1)
    qq3 = qq[:].rearrange("p (t c) -> p t c", c=8)

    # segmented reduces (DVE)
    rr3 = rr[:].rearrange("p (t c) -> p t c", c=8)
    TW = 2 * tf
    arena = small.tile([P, 8 * TW + tf], F32, tag="arena")
    savg = arena[:, 0 * TW : 1 * TW]
    qavg = arena[:, 1 * TW : 2 * TW]
    mq2 = arena[:, 2 * TW : 3 * TW]
    ps2 = arena[:, 3 * TW : 4 * TW]
    p2f = arena[:, 4 * TW : 5 * TW]
    u2 = arena[:, 5 * TW : 6 * TW]
    Dv2 = arena[:, 6 * TW : 7 * TW]  # also reused as vg scratch
    wf2 = arena[:, 7 * TW : 8 * TW]
    cf = arena[:, 8 * TW : 8 * TW + tf]
    nc.vector.tensor_reduce(savg, rr3, axis=X, op=op.add)
    if ENGINES["pe_qd"]:
        psum_pool2, ident2, nident2 = pools[3]
        NCH = 512
        qdp = psum_pool2.tile([P, FE], F32, tag="qdp")
        for c0 in range(0, FE, NCH):
            nc.tensor.matmul(qdp[:, c0:c0+NCH], ident2[:], qq[:, c0:c0+NCH], start=True, stop=False)
            nc.tensor.matmul(qdp[:, c0:c0+NCH], nident2[:], qq[:, FE+c0:FE+c0+NCH], start=False, stop=True)
        qdp3 = qdp[:].rearrange("p (t c) -> p t c", c=8)
        nc.vector.tensor_reduce(qavg[:, :tf], qdp3, axis=X, op=op.add)  # dS = S2_0 - S2_1
    else:
        nc.vector.tensor_reduce(qavg, qq3, axis=X, op=op.add)
    nc.vector.tensor_reduce(mq2, qq3, axis=X, op=op.max)

    # parity: ps = 2*round(s/2) - s in {-1,0,1}; p2f = ps^2 in {0,1}
    nc.vector.tensor_scalar(ps2, savg, 0.5, MAGIC, op0=op.mult, op1=op.add)
    nc.vector.tensor_scalar(ps2, ps2, MAGIC, None, op0=op.subtract)
    nc.vector.scalar_tensor_tensor(ps2, ps2, 2.0, savg, op0=op.mult, op1=op.subtract)
    nc.scalar.square(p2f, ps2)
    # u = 1 - 2*sqrt(mq)  (~1ulp sqrt; only perturbs borderline D compares)
    nc.scalar.sqrt(u2, mq2)
    nc.scalar.activation(
        u2, u2, mybir.ActivationFunctionType.Copy, bias=1.0, scale=-2.0
    )
    # D = S2 + p*u ; c = (D0 <= D1)
    nc.vector.tensor_tensor(Dv2, p2f, u2, op.mult)
    if ENGINES["pe_qd"]:
        nc.vector.tensor_tensor(Dv2[:, :tf], Dv2[:, :tf], qavg[:, :tf], op.add)
        nc.vector.tensor_tensor(cf, Dv2[:, :tf], Dv2[:, tf:], op.is_le)
    else:
        nc.vector.tensor_tensor(Dv2, Dv2, qavg, op.add)
        nc.vector.tensor_tensor(cf, Dv2[:, :tf], Dv2[:, tf:], op.is_le)
    # w0 = p0*c ; w1 = p1*(1-c); gated max-sq: mgq = mq*w + (w-1)
    nc.vector.tensor_tensor(wf2[:, :tf], p2f[:, :tf], cf, op.mult)
    nc.vector.scalar_tensor_tensor(
        wf2[:, tf:], cf, 1.0, p2f[:, tf:], op0=op.subtract, op1=op.mult
    )
    nc.vector.tensor_scalar(wf2[:, tf:], wf2[:, tf:], -1.0, None, op0=op.mult)
    nc.vector.tensor_tensor(Dv2, mq2, wf2, op.mult)
    nc.vector.tensor_scalar(wf2, wf2, 1.0, None, op0=op.subtract)
    nc.vector.tensor_tensor(mq2, Dv2, wf2, op.add)

    # onehot (GP, fp equality on squares); flip = sign(d) | onehot-bits (DVE)
    mgq_b = mq2.unsqueeze(2).broadcast_to([P, 2 * tf, 8])
    ohf = work.tile([P, 2 * FE], F32, tag="ohh")
    ohf3 = ohf[:].rearrange("p (t c) -> p t c", c=8)
    nc.vector.tensor_tensor(ohf3, qq3, mgq_b, op.is_equal)
    ohf_u = ohf[:].bitcast(U32)
    _stt_u32(nc.vector, ohf_u, dd_u[:], 0x80000000, ohf_u, op.bitwise_and, op.bitwise_or)
    fl = ohf[:]

    # d_sel = c ? d0 : d1 (ACT copy + DVE predicated); flsum on GP
    dsel = work.tile([P, 2 * FE], F32, tag="rr")
    nc.scalar.copy(dsel[:, :FE], d1)
    cI_b = cf.bitcast(U32).unsqueeze(2).broadcast_to([P, tf, 8])
    nc.vector.copy_predicated(
        dsel[:, :FE].rearrange("p (t c) -> p t c", c=8),
        cI_b,
        dd[:, :FE].rearrange("p (t c) -> p t c", c=8),
    )
    if ENGINES["pe_y"]:
        psum_pool, ident, nident = pools[3]
        yp = psum_pool.tile([P, FE], F32, tag="yp")
        NCH = 512
        for c0 in range(0, FE, NCH):
            sl = slice(c0, c0 + NCH)
            nc.tensor.matmul(yp[:, sl], ident[:], xt[:, sl], start=True, stop=False)
            nc.tensor.matmul(yp[:, sl], nident[:], dsel[:, c0:c0+NCH], start=False, stop=False)
            nc.tensor.matmul(yp[:, sl], ident[:], fl[:, c0:c0+NCH], start=False, stop=False)
            nc.tensor.matmul(yp[:, sl], ident[:], fl[:, FE+c0:FE+c0+NCH], start=False, stop=True)
        yt = stream.tile([P, FE], F32, tag="yt")
        nc.scalar.copy(yt[:], yp[:])
        nc.sync.dma_start(out=y_rows, in_=yt[:])
    else:
        E("flsum").tensor_tensor(dsel[:, FE:], fl[:, :FE], fl[:, FE:], op.add)
        yt = stream.tile([P, FE], F32, tag="yt")
        nc.vector.tensor_tensor(yt[:], xt[:], dsel[:, :FE], op.subtract)
        nc.vector.tensor_tensor(yt[:], yt[:], dsel[:, FE:], op.add)
        nc.sync.dma_start(out=y_rows, in_=yt[:])


def build_nc(shard=SHARD, tf=256, reps=1):
    P = 128
    pts = P * tf
    assert shard % pts == 0
    ntiles = shard // pts

    nc = bacc.Bacc("TRN2", target_bir_lowering=False, debug=False, num_devices=N_CORES)
    xd = nc.declare_dram_parameter("x", [shard, 8], F32, isOutput=False)
    yd = nc.declare_dram_parameter("y", [shard, 8], F32, isOutput=True)

    from concourse.masks import make_identity
    with TileContext(nc) as tc:
        with (
            tc.tile_pool(name="stream", bufs=2) as stream,
            tc.tile_pool(name="work", bufs=2) as work,
            tc.tile_pool(name="small", bufs=2) as small,
            tc.tile_pool(name="const", bufs=1) as cpool,
            tc.tile_pool(name="psum", bufs=2, space="PSUM") as psum_pool,
        ):
            pe = None
            if ENGINES["pe_y"]:
                ident = cpool.tile([P, P], F32, tag="ident")
                nident = cpool.tile([P, P], F32, tag="nident")
                make_identity(nc, ident[:])
                nc.scalar.activation(
                    nident[:], ident[:], mybir.ActivationFunctionType.Copy, scale=-1.0
                )
                pe = (psum_pool, ident, nident)
            for _ in range(reps):
                for t in range(ntiles):
                    _emit_tile(nc, (stream, work, small, pe), xd, yd, t, tf)
    nc.finalize()
    return nc


_BUILD_CACHE = {}


def _get_nc(shard, tf):
    key = (shard, tf)
    if key not in _BUILD_CACHE:
        _BUILD_CACHE[key] = build_nc(shard, tf)
    return _BUILD_CACHE[key]


def kernel(x: np.ndarray) -> np.ndarray:
    x = np.ascontiguousarray(x, dtype=np.float32)
    n = x.shape[0]
    shard = n // N_CORES
    tf = 256
    while shard % (128 * tf) != 0:
        tf //= 2
    nc = _get_nc(shard, tf)
    in_maps = [{"x": x[i * shard : (i + 1) * shard]} for i in range(N_CORES)]
    res = run_bass_kernel_spmd(nc, in_maps, list(range(N_CORES))).results
    return np.concatenate([res[i]["y"] for i in range(N_CORES)], axis=0)

